# revision 1
# baseline (speedup 1.0000x reference)
"""MoE gate routing kernel for Trainium2 (Bass/Tile), 8-way token-sharded.

Computes, for x = hidden_states.reshape(-1, H) and gate weight W [E, H]:
    logits = x @ W.T            # [T, E]
    top-8 of softmax(logits) with renormalized weights
Returns (topk_weight [T, 8] f32, topk_idx [T, 8] i32), matching the reference.

Math note: softmax then top-k + renormalize equals top-k on logits followed
by softmax over just those 8 logits (the global partition function cancels;
the reference's +1e-20 is negligible since the max prob >= 1/64).

Per-core schedule (2048 tokens = 4 slabs x 512 tokens):
  - W^T staged in SBUF once via PE transposes ([E,H] -> 32 tiles of [128,64]),
    in both f32 and f32r copies.
  - Per slab: DMA 4 x-blocks [128,4096] (contiguous rows); PE-transpose each
    [128,128] chunk into PSUM (4 token-blocks of one k-chunk share a bank),
    copy to SBUF (alternating DVE/ACT).
  - Contraction chunk pairs are computed either in f32r (two serial M=64
    matmuls into PSUM partitions 0:64; f32r streams the moving operand at
    ~1 cyc/row vs fp32's ~4, at ~1e-4 operand truncation) or in fp32
    (concurrent col-tiled pair: even chunk -> partitions 0:64 via
    tile_position (0,0), odd -> 64:128 via (0,64); exact). N_F32R=3 of the
    16 pairs use f32r - measured end-to-end rel err 1.30e-2 on topk_idx
    (vs the 2e-2 gate); larger N_F32R is faster but the idx error crosses
    the gate (all-f32r: 2.3e-2 at ~115us vs ~165us here).
  - Pipeline fixes over the previous revision: all x-load DMAs issue up
    front so the SP HWDGE ring is a pure x FIFO self-paced by pool WAR
    deps; the per-slab epilogue is emitted into the NEXT slab's chunk loop
    at spaced slots (producers >=2 slots ahead of cross-engine consumers)
    so strict-FIFO DVE/ACT queues never dam up; small epilogue ops run on
    the idle GpSimd engine; outputs accumulate into two [128,128] SBUF
    tiles and ship as 2 DMAs per rep (was 32 small ones, each costing ~1us
    of HWDGE ring occupancy); both halves of a pair copy PSUM->SBUF on one
    engine (per-kcp parity) so pair matmuls dispatch back-to-back.
  - A [128->64] merge matmul (two stacked identities) folds the fp32 odd-half
    into 0:64 (skipped when all pairs are f32r), then PE transposes logits^T
    back to [tokens, experts].
  - Top-8: DVE max8 + max-index; weights via ACT exp with per-row accumulate,
    DVE reciprocal and scale.
  - Periodic tiny bf16 matmuls keep the PE HAM clock-gate at 2.4 GHz.
"""

import numpy as np

import concourse.bass as bass
import concourse.mybir as mybir
from concourse import masks
from concourse.alu_op_type import AluOpType
from concourse.bass_utils import run_bass_kernel_spmd
from concourse.tile import TileContext

P = 128          # SBUF partitions
H = 4096         # hidden dim
E = 64           # experts
K = 8            # top-k
N_CORES = 8
T_TOTAL = 4 * 4096
T_CORE = T_TOTAL // N_CORES   # 2048
SLAB = 4 * P                  # 512 tokens per slab
N_SLAB = T_CORE // SLAB       # 4
N_KC = H // P                 # 32 contraction chunks
N_KCP = N_KC // 2             # 16 chunk pairs

N_F32R = 3                    # chunk pairs (of 16) computed in f32r

F32 = mybir.dt.float32
F16 = mybir.dt.float16
F32R = mybir.dt.float32r
BF16 = mybir.dt.bfloat16
U32 = mybir.dt.uint32
EXP = mybir.ActivationFunctionType.Exp


def build_bass(loop_reps=None, warm_every=1, xtp_bufs=4, lgt_bufs=1, xts_bufs=6,
               stage_depth=1, xin_bufs=8, n_f32r=N_F32R, copy_rot=2, ablate='full',
               outdma_eng='scalar', nmax_eng='gpsimd', w8_eng='gpsimd',
               mode='hybrid', xl_via='dve', tfold=False, force_xf32=False):
    fp16 = mode == 'fp16'
    comp = mode == 'comp'   # n_f32r pairs in f32r, rest fp16-split (exact)
    if comp:
        assert 0 < n_f32r < N_KCP
    all_r = (n_f32r >= N_KCP) and not fp16 and not comp
    xdt = F32R if (all_r and not force_xf32) else F32
    nc = bass.Bass()
    x = nc.declare_dram_parameter("x", [T_CORE, H], xdt, isOutput=False)
    w = nc.declare_dram_parameter("w", [E, H], F32, isOutput=False)
    # outputs packed [partition, slab*q*k]; kernel() un-permutes on host
    out_w = nc.declare_dram_parameter("out_w", [P, N_SLAB * 4 * K], F32, isOutput=True)
    out_i = nc.declare_dram_parameter("out_i", [P, N_SLAB * 4 * K], U32, isOutput=True)

    with TileContext(nc) as tc:
        with (
            tc.tile_pool(name="singles", bufs=1) as singles,
            tc.tile_pool(name="xin", bufs=xin_bufs) as x_pool,
            tc.tile_pool(name="xtp", bufs=xtp_bufs, space="PSUM") as xt_psum,
            tc.tile_pool(name="xts", bufs=xts_bufs) as xt_sb,
            tc.tile_pool(name="lgtp", bufs=lgt_bufs, space="PSUM") as lgt_psum,
            tc.tile_pool(name="mgp", bufs=1, space="PSUM") as mg_psum,
            tc.tile_pool(name="lgqp", bufs=1, space="PSUM") as lgq_psum,
            tc.tile_pool(name="wtp", bufs=1, space="PSUM") as wt_psum,
            tc.tile_pool(name="sm", bufs=3) as sm_pool,
            tc.tile_pool(name="oall", bufs=2) as o_pool,
        ):
            identity = singles.tile([P, P], F32)
            masks.make_identity(nc, identity[:])
            identity_x = identity
            if xdt != F32:
                identity_x = singles.tile([P, P], xdt)
                nc.vector.tensor_copy(identity_x[:], identity[:])
            # merge operand: [I64; I64] stacked -> adds PSUM halves
            mergeM = singles.tile([P, E], F32)
            nc.gpsimd.memset(mergeM[:], 0.0)
            masks.make_identity(nc, mergeM[0:E, 0:E], nomemset=True)
            masks.make_identity(nc, mergeM[E : 2 * E, 0:E], nomemset=True)
            # HAM warmers (tiny bf16 matmuls register PE activity)
            warm_l = singles.tile([P, 8], BF16)
            warm_r = singles.tile([P, 1], BF16)
            nc.vector.memset(warm_l[:], 0.0)
            nc.vector.memset(warm_r[:], 0.0)

            # W^T staging: wT[:, kc*E:(kc+1)*E] = W[:, kc*128:(kc+1)*128].T
            w_nat = singles.tile([E, H], F32)
            nc.sync.dma_start(out=w_nat[:], in_=w[:, :])
            # PE matmul weight-loads only support a single sem wait; this
            # absorber takes the gpsimd (identity) wait so the first real
            # transpose below only needs the DMA wait.
            absorber = wt_psum.tile([P, E], F32, tag="pw")
            nc.tensor.transpose(absorber[:E, :E], identity[:E, :E], identity[:E, :E])
            if fp16 or comp:
                # W split hi/lo fp16, packed [Wh | Wl] per chunk: W = Wh + Wl
                # to ~2^-22 relative; logits = xh(Wh+Wl) + xl*Wh (+O(2^-22))
                whl = singles.tile([P, N_KC * P], F16)
                if comp:
                    wTr = singles.tile([P, N_KC * E], F32R)
                for kc in range(N_KC):
                    pw = wt_psum.tile([P, E], F32, tag="pw")
                    nc.tensor.transpose(
                        pw[:], w_nat[:, kc * P : (kc + 1) * P], identity[:E, :E]
                    )
                    wh = whl[:, kc * P : kc * P + E]
                    nc.vector.tensor_copy(wh, pw[:])
                    nc.vector.tensor_tensor(
                        whl[:, kc * P + E : (kc + 1) * P], pw[:], wh,
                        AluOpType.subtract,
                    )
                    if comp:
                        nc.scalar.copy(wTr[:, kc * E : (kc + 1) * E], pw[:])
            else:
                wT = singles.tile([P, N_KC * E], F32)
                wTr = singles.tile([P, N_KC * E], F32R)
                for kc in range(N_KC):
                    pw = wt_psum.tile([P, E], F32, tag="pw")
                    nc.tensor.transpose(
                        pw[:], w_nat[:, kc * P : (kc + 1) * P], identity[:E, :E]
                    )
                    if kc % 2 == 0:
                        nc.vector.tensor_copy(wT[:, kc * E : (kc + 1) * E], pw[:])
                        nc.scalar.copy(wTr[:, kc * E : (kc + 1) * E], pw[:])
                    else:
                        nc.scalar.copy(wT[:, kc * E : (kc + 1) * E], pw[:])
                        nc.vector.tensor_copy(wTr[:, kc * E : (kc + 1) * E], pw[:])

            def issue_x(s):
                xb = []
                for tb in range(4):
                    t0 = s * SLAB + tb * P
                    xbt = x_pool.tile([P, H], xdt)
                    nc.sync.dma_start(out=xbt[:], in_=x[t0 : t0 + P, :])
                    xb.append(xbt)
                return xb

            def make_epi(s, lgt, w8all, i8all):
                """Epilogue for slab s as emission steps; interleaved into the
                next slab's chunk loop so DVE/ACT FIFO work never dams up in
                front of that slab's transpose copies."""
                state = {}

                def step_a():
                    if tfold and not all_r:
                        # fold mode: no merge matmul; just land lgt in SBUF
                        lgt_sb = sm_pool.tile([P, SLAB], F32, tag="lgt_sb")
                        nc.vector.tensor_copy(lgt_sb[:], lgt[:])
                        state["lgt_sb"] = lgt_sb
                        return
                    mg_sb = sm_pool.tile([E, SLAB], F32, tag="mg_sb")
                    if all_r:
                        nc.vector.tensor_copy(mg_sb[:], lgt[0:E, :])
                    else:
                        lgt_sb = sm_pool.tile([P, SLAB], F32, tag="lgt_sb")
                        nc.vector.tensor_copy(lgt_sb[:], lgt[:])
                        mg = mg_psum.tile([E, SLAB], F32, tag="epi")
                        nc.tensor.matmul(
                            mg[:], mergeM[:], lgt_sb[:], start=True, stop=True,
                            tile_position=(0, 0),
                        )
                        nc.vector.tensor_copy(mg_sb[:], mg[:])
                    state["mg_sb"] = mg_sb

                def step_b():
                    if tfold and not all_r:
                        # transpose both halves [128,512] -> tokens on
                        # partitions, then fold experts e and 64+e (now in the
                        # SAME partition) with free-dim adds - no merge matmul
                        lgt_sb = state["lgt_sb"]
                        lgq = lgq_psum.tile([P, 4 * P], F32, tag="lgq")
                        for q in range(4):
                            nc.tensor.matmul(
                                lgq[:, q * P : (q + 1) * P],
                                lgt_sb[:, q * P : (q + 1) * P],
                                identity[:, :],
                                is_transpose=True,
                                start=(q == 0),
                                stop=(q == 3),
                            )
                        lgf = sm_pool.tile([P, 4 * P], F32, tag="lgf")
                        nc.scalar.copy(lgf[:], lgq[:])
                        state["lgf"] = lgf
                        return
                    mg_sb = state["mg_sb"]
                    lgq = lgq_psum.tile([P, 4 * E], F32, tag="lgq")
                    for q in range(4):
                        nc.tensor.matmul(
                            lgq[:, q * E : (q + 1) * E],
                            mg_sb[:, q * P : (q + 1) * P],
                            identity[:E, :E],
                            is_transpose=True,
                            start=(q == 0),
                            stop=(q == 3),
                        )
                    lg_sb = sm_pool.tile([P, 4 * E], F32, tag="lg_sb")
                    nc.scalar.copy(lg_sb[:], lgq[:])
                    state["lg_sb"] = lg_sb

                def step_b2():
                    lgf = state["lgf"]
                    lg_sb = sm_pool.tile([P, 4 * E], F32, tag="lg_sb")
                    for q in range(4):
                        nc.vector.tensor_tensor(
                            lg_sb[:, q * E : (q + 1) * E],
                            lgf[:, q * P : q * P + E],
                            lgf[:, q * P + E : (q + 1) * P],
                            AluOpType.add,
                        )
                    state["lg_sb"] = lg_sb

                def make_max(q):
                    def step_max():
                        lg = state["lg_sb"][:, q * E : (q + 1) * E]
                        t8v = sm_pool.tile([P, K], F32, tag=f"t8v{q}")
                        nc.vector.max(out=t8v[:], in_=lg)
                        t8i = i8all[:, (s * 4 + q) * K : (s * 4 + q + 1) * K]
                        nc.vector.max_index(out=t8i, in_max=t8v[:], in_values=lg)
                        nmax = sm_pool.tile([P, 1], F32, tag=f"nmax{q}")
                        neng = nc.gpsimd if nmax_eng == "gpsimd" else nc.vector
                        neng.tensor_scalar_mul(nmax[:], t8v[:, 0:1], -1.0)
                        state[f"t8v{q}"] = t8v
                        state[f"t8i{q}"] = t8i
                        state[f"nmax{q}"] = nmax
                    return step_max

                def make_exp(q):
                    def step_exp():
                        e8 = sm_pool.tile([P, K], F32, tag=f"e8{q}")
                        s1 = sm_pool.tile([P, 1], F32, tag=f"s1{q}")
                        nc.scalar.activation(
                            e8[:], state[f"t8v{q}"][:], EXP,
                            bias=state[f"nmax{q}"][:], scale=1.0, accum_out=s1[:],
                        )
                        state[f"e8{q}"] = e8
                        state[f"s1{q}"] = s1
                    return step_exp

                def make_rcp(q):
                    def step_rcp():
                        r1 = sm_pool.tile([P, 1], F32, tag=f"r1{q}")
                        nc.vector.reciprocal(r1[:], state[f"s1{q}"][:])
                        state[f"r1{q}"] = r1
                    return step_rcp

                def make_fin(q):
                    def step_fin():
                        w8 = w8all[:, (s * 4 + q) * K : (s * 4 + q + 1) * K]
                        weng = nc.gpsimd if w8_eng == "gpsimd" else nc.vector
                        weng.tensor_scalar_mul(
                            w8, state[f"e8{q}"][:], state[f"r1{q}"][:, 0:1]
                        )
                    return step_fin

                if ablate == "epi_mg":
                    return [(0, step_a)]
                if ablate == "epi_lgq":
                    return [(0, step_a), (2, step_b)]
                steps = [(0, step_a), (1 if tfold else 2, step_b)]
                if tfold and not all_r:
                    steps += [(3, step_b2)]
                steps += [(4 + q, make_max(q)) for q in range(4)]
                steps += [(7 + q, make_exp(q)) for q in range(4)]
                steps += [(11 + q, make_rcp(q)) for q in range(4)]
                steps += [(13 + q, make_fin(q)) for q in range(4)]
                return steps

            def do_slab(s, xb, prev_epi, w8all, i8all):
                lgt = lgt_psum.tile([P, SLAB], F32)
                epi_steps = list(prev_epi) if prev_epi else []
                # One-stage software pipeline: transposes+copies for kcp are
                # emitted one step ahead of the consuming matmul pair.
                staged = []
                for kcp in range(N_KCP + stage_depth):
                    while epi_steps and epi_steps[0][0] <= kcp:
                        epi_steps.pop(0)[1]()
                    if kcp < N_KCP:
                        r_pair = kcp < n_f32r
                        cdt = F32R if r_pair else F32
                        split_pair = fp16 or (comp and not r_pair)
                        xt_s = []
                        for half in range(2):
                            kc = 2 * kcp + half
                            xt_p = xt_psum.tile([P, SLAB], xdt)
                            for tb in range(4):
                                nc.tensor.matmul(
                                    xt_p[:, tb * P : (tb + 1) * P],
                                    xb[tb][:, kc * P : (kc + 1) * P],
                                    identity_x[:],
                                    is_transpose=True,
                                    start=(tb == 0),
                                    stop=(tb == 3),
                                )
                            if split_pair:
                                xh = xt_sb.tile([P, SLAB], F16, tag="xh")
                                nc.scalar.copy(xh[:], xt_p[:])
                                xl = xt_sb.tile([P, SLAB], F16, tag="xl")
                                if xl_via == "copy":
                                    # timing probe only - WRONG numerics
                                    nc.vector.tensor_copy(xl[:], xt_p[:])
                                elif xl_via == "gpsimd":
                                    nc.gpsimd.tensor_tensor(
                                        xl[:], xt_p[:], xh[:], AluOpType.subtract
                                    )
                                else:
                                    nc.vector.tensor_tensor(
                                        xl[:], xt_p[:], xh[:], AluOpType.subtract
                                    )
                                xt_s.append((xh, xl))
                                continue
                            xt = xt_sb.tile([P, SLAB], cdt)
                            if copy_rot == 0:
                                # split copy: DVE and ACT each move half, in
                                # parallel - halves the PSUM->SBUF latency the
                                # consuming matmul waits on
                                hw = SLAB // 2
                                nc.vector.tensor_copy(xt[:, 0:hw], xt_p[:, 0:hw])
                                nc.scalar.copy(xt[:, hw:SLAB], xt_p[:, hw:SLAB])
                            else:
                                # per-kcp parity: both halves of a pair copy
                                # on ONE engine so the pair can dispatch
                                # back-to-back (required for array overlap)
                                eng = kcp % copy_rot
                                if eng == 0:
                                    nc.vector.tensor_copy(xt[:], xt_p[:])
                                elif eng == 1:
                                    nc.scalar.copy(xt[:], xt_p[:])
                                else:
                                    nc.gpsimd.tensor_copy(xt[:], xt_p[:])
                            xt_s.append(xt)
                        staged.append((kcp, xt_s))
                    if len(staged) > stage_depth or (
                        kcp >= N_KCP and staged
                    ):
                        pkcp, pxt = staged.pop(0)
                        if ablate in ("no_mm",):
                            continue
                        r_pair = pkcp < n_f32r
                        first = pkcp == 0
                        last = pkcp == N_KCP - 1
                        if fp16 or (comp and not r_pair):
                            kc_f16_first = 2 * n_f32r if comp else 0
                            for half in range(2):
                                kc = 2 * pkcp + half
                                xh, xl = pxt[half]
                                klast = kc == N_KC - 1
                                kfirst = comp and kc == kc_f16_first
                                def mm1():
                                    nc.tensor.matmul(
                                        lgt[:, :],
                                        whl[:, kc * P : (kc + 1) * P],
                                        xh[:],
                                        start=(kc == 0 and not comp),
                                        stop=klast,
                                        tile_position=(0, 0),
                                        skip_group_check=True,
                                    )
                                def mm2():
                                    # comp: first fp16 chunk's mm2 claims the
                                    # 64:128 region with start=True (the f32r
                                    # pairs only ever touched 0:64)
                                    pos = E if kfirst else (kc % 2) * E
                                    nc.tensor.matmul(
                                        lgt[pos : pos + E, :],
                                        whl[:, kc * P : kc * P + E],
                                        xl[:],
                                        start=kfirst, stop=False,
                                        tile_position=(0, pos),
                                        skip_group_check=True,
                                    )
                                if klast or kfirst:
                                    mm2(); mm1()
                                else:
                                    mm1(); mm2()
                            if pkcp % warm_every == 0:
                                wp = wt_psum.tile([8, 1], F32, tag="pw")
                                nc.tensor.matmul(
                                    wp[:8, :1], warm_l[:], warm_r[:],
                                    start=True, stop=True,
                                )
                            continue
                        if r_pair:
                            # two serial f32r matmuls, both into partitions 0:64
                            for half in range(2):
                                kc = 2 * pkcp + half
                                nc.tensor.matmul(
                                    lgt[0:E, :],
                                    wTr[:, kc * E : (kc + 1) * E],
                                    pxt[half][:],
                                    start=(first and half == 0),
                                    stop=(last and half == 1),
                                    tile_position=(0, 0),
                                )
                        else:
                            # concurrent col-tiled fp32 pair: even kc ->
                            # partitions 0:64, odd kc -> 64:128.  start=True
                            # must fire on the FIRST matmul touching each
                            # PSUM partition region: 0:64 is started by pair
                            # 0 (f32r or fp32), 64:128 by the first fp32 pair.
                            for half in range(2):
                                kc = 2 * pkcp + half
                                region_first = (
                                    first if half == 0 else pkcp == n_f32r
                                )
                                nc.tensor.matmul(
                                    lgt[half * E : (half + 1) * E, :],
                                    wT[:, kc * E : (kc + 1) * E],
                                    pxt[half][:],
                                    start=region_first,
                                    stop=last,
                                    tile_position=(0, half * E),
                                )
                        if pkcp % warm_every == 0:
                            wp = wt_psum.tile([8, 1], F32, tag="pw")
                            nc.tensor.matmul(
                                wp[:8, :1], warm_l[:], warm_r[:],
                                start=True, stop=True,
                            )

                # drain any leftover epilogue steps of the previous slab
                while epi_steps:
                    epi_steps.pop(0)[1]()
                if ablate in ("no_mm", "no_epi"):
                    return []
                return make_epi(s, lgt, w8all, i8all)

            def main_body():
                # All x-load DMAs issue up front: the SP HWDGE ring is then a
                # pure x-load FIFO; each DMA self-paces on its pool slot's WAR
                # dependency (xin_bufs=8 -> two slabs in flight).
                xbs = [issue_x(s) for s in range(N_SLAB)]
                if ablate == "dma_only":
                    return
                w8all = o_pool.tile([P, N_SLAB * 4 * K], F32, tag="w8all")
                i8all = o_pool.tile([P, N_SLAB * 4 * K], U32, tag="i8all")
                epi = None
                for s in range(N_SLAB):
                    epi = do_slab(s, xbs[s], epi, w8all, i8all)
                if ablate == "no_epi":
                    return
                for _, step in epi:
                    step()
                if ablate == "epi_top8":
                    return
                # two output stores per rep, on the ACT HWDGE ring (off the
                # SP ring's x-load FIFO); subtile deps gate on all 32 writes
                nc.scalar.dma_start(out=out_w[:, :], in_=w8all[:])
                nc.scalar.dma_start(out=out_i[:, :], in_=i8all[:])

            if loop_reps is None:
                main_body()
            else:
                with tc.For_i(0, loop_reps, 1):
                    main_body()

    _legalize_waits(nc)
    return nc


def _legalize_waits(nc):
    """Walrus allows only one sem wait on most instruction structs (matmul
    weight-load, DVE/ACT compute, pseudo-DMA, drain). Tile sometimes emits
    more. Fix: hoist excess waits onto standalone EventSemaphore instructions
    inserted just before the owner in its engine stream (same engine ->
    in-order issue preserves semantics)."""
    n = 0
    for f in nc.m.functions:
        for blk in f.blocks:
            out = []
            changed = False
            for i in blk.instructions:
                si = getattr(i, "sync_info", None)
                ow = list(si.on_wait) if (si is not None and si.on_wait) else []
                if len(ow) > 1:
                    while len(ow) > 1:
                        w = ow.pop(0)
                        out.append(
                            mybir.InstEventSemaphore(
                                name=f"I-whoist-{n}",
                                engine=i.engine,
                                ins=[],
                                outs=[],
                                sync_info=mybir.SyncInfo(on_wait=[w], on_update=[]),
                            )
                        )
                        n += 1
                    si.on_wait = ow
                    changed = True
                out.append(i)
            if changed:
                blk.instructions = out
    return nc


_NC = None


def _get_nc():
    global _NC
    if _NC is None:
        _NC = build_bass()
    return _NC


def kernel(hidden_states, weight, **run_kwargs):
    hs = np.ascontiguousarray(np.asarray(hidden_states, dtype=np.float32)).reshape(
        T_TOTAL, H
    )
    wt = np.ascontiguousarray(np.asarray(weight, dtype=np.float32))
    nc = _get_nc()
    in_maps = [
        {"x": np.ascontiguousarray(hs[i * T_CORE : (i + 1) * T_CORE]), "w": wt}
        for i in range(N_CORES)
    ]
    res = run_bass_kernel_spmd(nc, in_maps, core_ids=list(range(N_CORES)), **run_kwargs)

    def unpack(a):
        # [P, N_SLAB*4*K] with column (s*4+q)*K+k holding token s*512+q*128+p
        return (
            a.reshape(P, N_SLAB, 4, K).transpose(1, 2, 0, 3).reshape(T_CORE, K)
        )

    topk_weight = np.concatenate([unpack(r["out_w"]) for r in res.results], axis=0)
    topk_idx = np.concatenate(
        [unpack(r["out_i"]).astype(np.int32) for r in res.results], axis=0
    )
    if run_kwargs:
        kernel.last_result = res
    return topk_weight, topk_idx

